# revision 4
# baseline (speedup 1.0000x reference)
"""Lorenz-96 vector field kernel for Trainium2 (8 NeuronCores, SPMD data-parallel).

field[..., i] = p0[i]*(state[i+1] - state[i-2])*state[i-1] - p1[i]*state[i] + p2[i]
(circular along the last axis, dim=256)

Sharding: batch axis (262144 rows) split evenly across 8 cores; params replicated.

Per-core layout: each SBUF partition holds R batch rows as one flat stream of
R*259 floats: every row is [halo2 | 256 cols | halo1] where the 3-wide halo
carries the circular wrap (s[254], s[255] on the left, s[0] on the right).
All shifted stencil operands are then contiguous *flat 2D* views of the stream
(offset +-1/+-2), so every tensor_tensor op uses the 2D S2S2D2 ISA encoding
(the 3D S3S3D3_TT struct has no room for multiple semaphore waits and fails
walrus codegen). Halo lanes compute garbage that is never stored - the output
DMA reads only the 256 real columns per row.

Engine split: 4 fp32 tensor_tensor ops on VectorE + 2 on GPSIMD (~2:1 rate
ratio) to approach the HBM roofline; ScalarE does the tiny halo fills.
"""

import numpy as np

import concourse.bass as bass
import concourse.mybir as mybir
from concourse.tile import TileContext
from concourse.bass_utils import run_bass_kernel_spmd
from concourse.vector_clock import ScopedClock, VectorClock


class SplitDrainTileContext(TileContext):
    """The kernel-tail Drain aggregates one sem wait per outstanding proc
    (compute engines + every HWDGE queue used); walrus rejects instructions
    with more than a couple of encoded waits. Pre-observe each proc with its
    own single-wait SP nop so the real drain needs none."""

    def _drain_and_barrier(self, tick_clock, wait_clock):
        full = tick_clock.global_clock
        n = len(list(full))
        for p in range(n):
            if full[p] == 0:
                continue
            partial = VectorClock([full[q] if q == p else 0 for q in range(n)])
            nop = self.nc.sync.nop(nofuse=True)
            wait_clock.add_sem_waits(nop.ins, ScopedClock({None: partial}))
        # All outstanding work is observed by the in-order SP nops above, so
        # the drain itself needs no encoded waits (walrus caps them at ~4).
        self.nc.sync.drain()
        self.nc.all_engine_barrier()
        assert self.sems is not None
        popped = self.nc._tile_sem_poison_stack.pop()
        assert popped is self._sem_poison
        self.nc.clear_and_free_semaphores(list(self.sems.allocated().values()))
        self.nc.all_engine_barrier()

def _split_waits(nc, limit: int = 1):
    """Post-lowering pass: walrus caps encoded sem waits per instruction
    (TT allows 1, DMACopy ~2). Move excess waits onto same-engine NoOps
    inserted immediately before the instruction - sequencers issue in
    order, so waiting earlier on the same stream preserves ordering."""
    for bb in nc.m.functions[0].blocks:
        il = bb.instructions
        i = 0
        while i < len(il):
            ins = il[i]
            si = getattr(ins, "sync_info", None)
            if si is not None and len(si.on_wait) > limit:
                waits = list(si.on_wait)
                keep, excess = waits[-limit:], waits[:-limit]
                for j, w in enumerate(excess):
                    nop = mybir.InstNoOp(
                        name=f"{ins.name}-wsplit{j}", ins=[], outs=[]
                    )
                    nop.engine = ins.engine
                    nop.sync_info = mybir.SyncInfo(on_wait=[w], on_update=[])
                    il.insert(i, nop)
                    i += 1
                ins.sync_info = mybir.SyncInfo(on_wait=keep, on_update=si.on_update)
            i += 1


P = 128          # SBUF partitions
DIM = 256        # Lorenz-96 dimension (stencil axis, unsharded)
EXT = DIM + 3    # per-row stream width incl. halo
NCORES = 8
R = 8            # batch rows per partition per tile
F32 = mybir.dt.float32


def build_nc(rows: int, r: int = R, repeat: int = 1):
    """Build the per-core Bass program. `rows` = batch rows per core.
    `repeat` re-runs the full pass (bench-only knob for slope timing)."""
    assert rows % (P * r) == 0
    nt = rows // (P * r)
    W = r * EXT          # flat stream width per partition
    G0, G1 = 2, W - 1    # compute range (shifts -2..+1 stay in bounds)
    FD = G1 - G0

    nc = bass.Bass()
    st = nc.declare_dram_parameter("state", [rows, DIM], F32, isOutput=False)
    pb = nc.declare_dram_parameter("pb", [P, 3, W], F32, isOutput=False)
    out = nc.declare_dram_parameter("out", [rows, DIM], F32, isOutput=True)

    st_t = st.rearrange("(n p r) d -> n p r d", p=P, r=r)
    out_t = out.rearrange("(n p r) d -> n p r d", p=P, r=r)

    with SplitDrainTileContext(nc) as tc:
        with (
            tc.tile_pool(name="pp", bufs=1) as ppool,
            tc.tile_pool(name="ext", bufs=4) as extpool,
            tc.tile_pool(name="mid", bufs=3) as midpool,
            tc.tile_pool(name="op", bufs=4) as opool,
        ):
            pbt = ppool.tile([P, 3 * W], F32)
            nc.sync.dma_start(out=pbt[:], in_=pb.rearrange("p a w -> p (a w)"))
            P0 = pbt[:, 0 * W + G0 : 0 * W + G1]
            P1 = pbt[:, 1 * W + G0 : 1 * W + G1]
            P2 = pbt[:, 2 * W + G0 : 2 * W + G1]

            # dep-collector warmups: both compute engines observe the pbt DMA
            # here so loop ops never carry a pbt wait (TT encodings allow only
            # ONE sync-wait slot). Every collector writes its own scratch
            # column - overlapping writes on Pool would add a self-sem wait.
            wu = ppool.tile([P, 8 + 2 * nt * repeat], F32)
            nc.gpsimd.tensor_copy(wu[:, 0:1], pbt[:, 0:1])
            nc.vector.tensor_copy(wu[:, 4:5], pbt[:, 0:1])

            for i in range(nt * repeat):
                ext = extpool.tile([P, W], F32, tag="ext")
                e3 = ext[:].rearrange("p (r c) -> p r c", c=EXT)
                nc.sync.dma_start(out=e3[:, :, 2 : DIM + 2], in_=st_t[i % nt])
                # halo fill on VectorE (same engine as half the consumers →
                # no extra semaphore): left 2 cols = state[254:256], right = state[0]
                nc.vector.tensor_copy(e3[:, :, 0:2], e3[:, :, DIM : DIM + 2])
                nc.vector.tensor_copy(e3[:, :, DIM + 2 : DIM + 3], e3[:, :, 2:3])

                A = ext[:, G0:G1]            # s[c]
                Am1 = ext[:, G0 - 1 : G1 - 1]  # s[c-1]
                Am2 = ext[:, G0 - 2 : G1 - 2]  # s[c-2]
                Ap1 = ext[:, G0 + 1 : G1 + 1]  # s[c+1]

                um1 = midpool.tile([P, W], F32, tag="um1")
                diff = midpool.tile([P, W], F32, tag="diff")
                vt = midpool.tile([P, W], F32, tag="v")
                ot = opool.tile([P, W], F32, tag="o")

                # dep-collectors: TT instructions encode at most ONE sem wait,
                # and the GPSIMD TT ops below depend on both the ext DMA and
                # the VectorE halo fill. These two copies each carry one wait,
                # after which the TT ops need none (sequencer-order suffices).
                c0 = 8 + 2 * i
                nc.gpsimd.tensor_copy(wu[:, c0 : c0 + 1], ext[:, 2:3])
                nc.gpsimd.tensor_copy(wu[:, c0 + 1 : c0 + 2], ext[:, 0:1])

                # um1[c] = p0[c] * s[c-1]   (GPSIMD)
                nc.gpsimd.tensor_mul(um1[:, G0:G1], Am1, P0)
                # diff[c] = s[c+1] - s[c-2] (GPSIMD)
                nc.gpsimd.tensor_sub(diff[:, G0:G1], Ap1, Am2)
                # v[c] = p1[c] * s[c]
                nc.vector.tensor_mul(vt[:, G0:G1], A, P1)
                # z = diff * um1   (in-place into um1)
                nc.vector.tensor_mul(um1[:, G0:G1], diff[:, G0:G1], um1[:, G0:G1])
                # f = z - v        (in-place into um1)
                nc.vector.tensor_sub(um1[:, G0:G1], um1[:, G0:G1], vt[:, G0:G1])
                # out = f + p2
                nc.vector.tensor_add(ot[:, G0:G1], um1[:, G0:G1], P2)

                o3 = ot[:].rearrange("p (r c) -> p r c", c=EXT)
                nc.sync.dma_start(out=out_t[i % nt], in_=o3[:, :, 2 : DIM + 2])

    _split_waits(nc)
    return nc


def make_pb(params: np.ndarray, r: int = R) -> np.ndarray:
    """Host-side param prep: 259-periodic stream, tiled r times, bcast to 128."""
    row = np.zeros((3, EXT), np.float32)
    row[:, 2 : DIM + 2] = params
    stream = np.tile(row, (1, r))  # [3, r*EXT]
    return np.ascontiguousarray(np.broadcast_to(stream[None], (P, 3, r * EXT)))


_cache: dict = {}


def _get_nc(rows: int):
    if rows not in _cache:
        _cache[rows] = build_nc(rows)
    return _cache[rows]


def kernel(state: np.ndarray, params: np.ndarray, t: np.ndarray = None) -> np.ndarray:
    state = np.ascontiguousarray(state, dtype=np.float32)
    params = np.asarray(params, dtype=np.float32)
    B = state.shape[0]
    rows = B // NCORES
    nc = _get_nc(rows)
    pb = make_pb(params)
    in_maps = [
        {"state": state[i * rows : (i + 1) * rows], "pb": pb} for i in range(NCORES)
    ]
    res = run_bass_kernel_spmd(nc, in_maps, list(range(NCORES)))
    return np.concatenate([res.results[i]["out"] for i in range(NCORES)], axis=0)



# revision 5
# speedup vs baseline: 15.9394x; 15.9394x over previous
"""Lorenz-96 vector field kernel for Trainium2 (8 NeuronCores, SPMD data-parallel).

field[..., i] = p0[i]*(state[i+1] - state[i-2])*state[i-1] - p1[i]*state[i] + p2[i]
(circular along the last axis, dim=256)

Sharding: batch axis (262144 rows) split evenly across 8 cores; params replicated.

Per-core design (arrived at empirically on TRN2):
  - DVE (VectorE) and GPSIMD tensor_tensor ops serialize against each other
    on the shared SBUF port pair, so GPSIMD is not used at all; every math op
    runs on DVE. ACT (ScalarE) has dedicated SBUF ports and overlaps DVE
    fully, so it carries all casts/halo fills and issues the output DMAs.
  - DVE fp32 TT = 1 elem/cycle/lane, but bf16 TT with unit step and
    4B-aligned operand starts runs 2x packed. All six TT ops run bf16 2x.

Layout: per partition, r=8 batch rows, each row a 260-wide bf16 stream
[haloL(2) | 256 cols | haloR(1) | pad(1)] - the even row stride keeps every
row start 4B-aligned. The stencil needs both parities (shifts -2,-1,0,+1):
ACT materializes the stream ebf AND a one-element-shifted copy ex1
(ex1[t] = ebf[t+1]), so every DVE operand starts on an even element:
    um1 = ex1[j-2] * P0[j]        (= s[j-1]*p0[j])
    df  = ex1[j]   - ebf[j-2]     (= s[j+1]-s[j-2])
    vt  = ebf[j]   * P1[j]        (= s[j]*p1[j])
    um1 = df * um1 ; um1 = um1 - vt ; df = um1 + P2[j]   (all bf16 2x)
ACT per tile: castin f32->bf16 into the halo layout, halo fills, shift copy;
then (la tiles later) castout bf16->f32 and the output DMA - the DMA is
HWDGE on the ACT ring and issues in order right after its castout, so it
needs no extra semaphores and never head-of-line blocks the next tile's
casts. Input DMA on the SP ring. Both DRAM transfers are fully contiguous
8KB per partition. bf16 keeps rel err ~4e-3, well inside the 2e-2 gate.
"""

import numpy as np
import ml_dtypes

import concourse.bass as bass
import concourse.mybir as mybir
from concourse.tile import TileContext
from concourse.bass_utils import run_bass_kernel_spmd
from concourse.vector_clock import ScopedClock, VectorClock


class SplitDrainTileContext(TileContext):
    """The kernel-tail Drain aggregates one sem wait per outstanding proc
    (compute engines + every HWDGE queue used); walrus rejects instructions
    with more than a couple of encoded waits. Pre-observe each proc with its
    own single-wait SP nop so the real drain needs none."""

    def _drain_and_barrier(self, tick_clock, wait_clock):
        full = tick_clock.global_clock
        n = len(list(full))
        for p in range(n):
            if full[p] == 0:
                continue
            partial = VectorClock([full[q] if q == p else 0 for q in range(n)])
            nop = self.nc.sync.nop(nofuse=True)
            wait_clock.add_sem_waits(nop.ins, ScopedClock({None: partial}))
        # All outstanding work is observed by the in-order SP nops above, so
        # the drain itself needs no encoded waits (walrus caps them at ~4).
        self.nc.sync.drain()
        self.nc.all_engine_barrier()
        assert self.sems is not None
        popped = self.nc._tile_sem_poison_stack.pop()
        assert popped is self._sem_poison
        self.nc.clear_and_free_semaphores(list(self.sems.allocated().values()))
        self.nc.all_engine_barrier()


def _split_waits(nc, limit: int = 1):
    """Post-lowering pass: walrus caps encoded sem waits per instruction
    (TT allows 1, DMACopy ~2). Move excess waits onto same-engine NoOps
    inserted immediately before the instruction - sequencers issue in
    order, so waiting earlier on the same stream preserves ordering."""
    for bb in nc.m.functions[0].blocks:
        il = bb.instructions
        i = 0
        while i < len(il):
            ins = il[i]
            si = getattr(ins, "sync_info", None)
            if si is not None and len(si.on_wait) > limit:
                waits = list(si.on_wait)
                keep, excess = waits[-limit:], waits[:-limit]
                for j, w in enumerate(excess):
                    nop = mybir.InstNoOp(
                        name=f"{ins.name}-wsplit{j}", ins=[], outs=[]
                    )
                    nop.engine = ins.engine
                    nop.sync_info = mybir.SyncInfo(on_wait=[w], on_update=[])
                    il.insert(i, nop)
                    i += 1
                ins.sync_info = mybir.SyncInfo(on_wait=keep, on_update=si.on_update)
            i += 1


P = 128          # SBUF partitions
DIM = 256        # Lorenz-96 dimension (stencil axis, unsharded)
EXT = DIM + 3    # (legacy 259-layout constant, kept for tooling imports)
EXT2 = DIM + 4   # per-row bf16 stream width: 2 halo + 256 + 1 halo + 1 pad
NCORES = 8
R = 8            # batch rows per partition per tile
F32 = mybir.dt.float32
BF16 = mybir.dt.bfloat16


def build_nc(rows: int, r: int = R, repeat: int = 1,
             ebufs: int = 3, mbufs: int = 3, obufs: int = 3, la: int = 2):
    """Build the per-core Bass program. `rows` = batch rows per core.
    `repeat` re-runs the full pass (bench-only knob for slope timing)."""
    assert rows % (P * r) == 0
    nt = rows // (P * r)
    W = r * EXT2            # bf16 stream width per partition
    C = r * DIM             # contiguous f32 cols per partition
    v0 = 2
    NV = W - 4              # op width (covers all real outputs; even)

    nc = bass.Bass()
    st = nc.declare_dram_parameter("state", [rows, DIM], F32, isOutput=False)
    pb = nc.declare_dram_parameter("pb", [P, 3, W], BF16, isOutput=False)
    out = nc.declare_dram_parameter("out", [rows, DIM], F32, isOutput=True)

    st_t = st.rearrange("(n p r) d -> n p (r d)", p=P, r=r)
    out_t = out.rearrange("(n p r) d -> n p (r d)", p=P, r=r)

    with SplitDrainTileContext(nc) as tc:
        with (
            tc.tile_pool(name="pp", bufs=1) as ppool,
            tc.tile_pool(name="in32", bufs=ebufs) as inpool,
            tc.tile_pool(name="bfs", bufs=ebufs) as bfpool,
            tc.tile_pool(name="mid", bufs=mbufs) as midpool,
            tc.tile_pool(name="o32", bufs=obufs) as opool,
        ):
            pbt = ppool.tile([P, 3 * W], BF16)
            nc.sync.dma_start(out=pbt[:], in_=pb.rearrange("p a w -> p (a w)"))

            def PB(a):
                return pbt[:, a * W + v0 : a * W + v0 + NV]

            # one-time pbt observation on V (dep collector: loop TT ops then
            # never carry a pbt wait; TT encodings allow one sync-wait slot)
            wu = ppool.tile([P, 8], BF16)
            nc.vector.tensor_copy(wu[:, 0:1], pbt[:, 0:1])

            iters = nt * repeat
            st8 = [None] * iters   # (dfv, of32) per tile

            def emit_tail(j):
                dfv, of32 = st8[j]
                o3 = dfv.rearrange("p (r c) -> p r c", c=EXT2)
                oc = of32.rearrange("p (r c) -> p r c", c=DIM)
                nc.scalar.copy(oc[:], o3[:, :, 0:DIM])         # castout bf16->f32
                nc.scalar.dma_start(out=out_t[j % nt], in_=of32)

            for i in range(iters):
                e32 = inpool.tile([P, C], F32, tag="e32")
                nc.sync.dma_start(out=e32[:], in_=st_t[i % nt])

                ebf = bfpool.tile([P, W], BF16, tag="ebf")
                ex1 = bfpool.tile([P, W], BF16, tag="ex1")
                eb3 = ebf[:].rearrange("p (r c) -> p r c", c=EXT2)
                ec3 = e32[:].rearrange("p (r c) -> p r c", c=DIM)
                # castin f32 -> bf16 into the halo layout
                nc.scalar.copy(eb3[:, :, 2 : DIM + 2], ec3[:])
                # halo fills: left 2 cols = s[254:256]
                nc.scalar.copy(eb3[:, :, 0:2], eb3[:, :, DIM : DIM + 2])
                # right halo + pad in one op: cols 258,259 := s[0], s[1]
                # (pad only feeds garbage lanes; s[1] is as good as s[0])
                nc.scalar.copy(eb3[:, :, DIM + 2 : DIM + 4], eb3[:, :, 2:4])
                # shifted parity copy: ex1[t] = ebf[t+1]
                nc.scalar.copy(ex1[:, 0 : W - 2], ebf[:, 1 : W - 1])

                if i >= la:
                    emit_tail(i - la)

                um1 = midpool.tile([P, W], BF16, tag="um1")
                dfv = midpool.tile([P, W], BF16, tag="dfv")
                vtv = midpool.tile([P, W], BF16, tag="vtv")
                of32 = opool.tile([P, C], F32, tag="of32")

                nc.vector.tensor_mul(um1[:, 0:NV], ex1[:, v0 - 2 : v0 - 2 + NV], PB(0))
                nc.vector.tensor_sub(dfv[:, 0:NV], ex1[:, v0 : v0 + NV], ebf[:, v0 - 2 : v0 - 2 + NV])
                nc.vector.tensor_mul(vtv[:, 0:NV], ebf[:, v0 : v0 + NV], PB(1))
                nc.vector.tensor_mul(um1[:, 0:NV], dfv[:, 0:NV], um1[:, 0:NV])
                nc.vector.tensor_sub(um1[:, 0:NV], um1[:, 0:NV], vtv[:, 0:NV])
                nc.vector.tensor_add(dfv[:, 0:NV], um1[:, 0:NV], PB(2))

                st8[i] = (dfv[:], of32[:])

            for j in range(max(0, iters - la), iters):
                emit_tail(j)

    _split_waits(nc)
    return nc


def make_pb(params: np.ndarray, r: int = R) -> np.ndarray:
    """bf16 param streams, 260-periodic: [p[254],p[255],p[0..255],p[0],p[0]],
    replicated r times and broadcast across the 128 partitions."""
    row = np.zeros((3, EXT2), np.float32)
    row[:, 2 : DIM + 2] = params
    row[:, 0:2] = params[:, DIM - 2 : DIM]
    row[:, DIM + 2] = params[:, 0]
    row[:, DIM + 3] = params[:, 0]
    stream = np.tile(row, (1, r)).astype(ml_dtypes.bfloat16)
    return np.ascontiguousarray(np.broadcast_to(stream[None], (P, 3, r * EXT2)))


_cache: dict = {}


def _get_nc(rows: int):
    if rows not in _cache:
        _cache[rows] = build_nc(rows)
    return _cache[rows]


def kernel(state: np.ndarray, params: np.ndarray, t: np.ndarray = None) -> np.ndarray:
    state = np.ascontiguousarray(state, dtype=np.float32)
    params = np.asarray(params, dtype=np.float32)
    B = state.shape[0]
    rows = B // NCORES
    nc = _get_nc(rows)
    pb = make_pb(params)
    in_maps = [
        {"state": state[i * rows : (i + 1) * rows], "pb": pb} for i in range(NCORES)
    ]
    res = run_bass_kernel_spmd(nc, in_maps, list(range(NCORES)))
    return np.concatenate([res.results[i]["out"] for i in range(NCORES)], axis=0)
